# revision 1
# baseline (speedup 1.0000x reference)
"""Trainium2 Bass kernel for nn_Activity_Detection: 3-modality bidirectional
LSTM activity head.

Sharding (8 NeuronCores): 4 batch shards (128 rows) x 2 LSTM directions.
Cores 0-3 run the forward LSTMs, cores 4-7 the reverse LSTMs on host
time-reversed features; one SPMD program. Each core computes, per modality,
projT = (x @ W.T + b).T on the PE (audio's 128-wide projection is folded into
its gate weights on the host), then a 20-step LSTM recurrence in bf16 with
fp32 PSUM accumulation, and finally a partial output
(h_a*h_r*h_c) @ W_out_half.T in fp32. The host sums fwd+rev partials + b_out.
"""

import numpy as np
import ml_dtypes

import concourse.bass as bass
import concourse.bacc as bacc
import concourse.tile as tile
import concourse.mybir as mybir
from concourse.masks import make_identity
from concourse import bass_utils

BF16 = mybir.dt.bfloat16
F32 = mybir.dt.float32
AF = mybir.ActivationFunctionType

B, T = 512, 20
RES, C3D, AUD, P, H, NCLS = 2048, 4096, 128, 1024, 1024, 200
BS = 128          # batch rows per core
G4 = 4 * H        # 4096 gate dim
NKH = H // 128    # 8 h chunks
MODS = ("audio", "resnet", "c3d")
DIMS = {"audio": AUD, "resnet": RES, "c3d": C3D}

TRACE = False            # set by test harness for profiling
LAST_RESULTS = None      # BassKernelResults of the last run (for profiling)


def build_program(has_gate_bias: bool):
    nc = bacc.Bacc("TRN2", target_bir_lowering=False, debug=False, num_devices=1)

    x_d = {m: nc.dram_tensor(f"x_{m}", [T, BS, DIMS[m]], BF16, kind="ExternalInput").ap()
           for m in MODS}
    wt_d = {m: nc.dram_tensor(f"wt_{m}", [DIMS[m], P], BF16, kind="ExternalInput").ap()
            for m in ("resnet", "c3d")}
    bp_d = {m: nc.dram_tensor(f"bp_{m}", [P], F32, kind="ExternalInput").ap()
            for m in ("resnet", "c3d")}
    kd = {"audio": AUD + H, "resnet": P + H, "c3d": P + H}
    ww_d = {m: nc.dram_tensor(f"ww_{m}", [kd[m], G4], BF16, kind="ExternalInput").ap()
            for m in MODS}
    gb_d = {m: nc.dram_tensor(f"gb_{m}", [G4], F32, kind="ExternalInput").ap()
            for m in MODS}
    wout_d = nc.dram_tensor("wout", [H, NCLS], F32, kind="ExternalInput").ap()
    out_d = nc.dram_tensor("out_partial", [BS, NCLS], F32, kind="ExternalOutput").ap()
    pjt_d = {m: nc.dram_tensor(f"pjt_{m}", [T, NKH, 128, BS], BF16, kind="Internal").ap()
             for m in ("resnet", "c3d")}

    from contextlib import ExitStack
    with tile.TileContext(nc) as tc, ExitStack() as stack:
        const = stack.enter_context(tc.tile_pool(name="const", bufs=1))
        psum = stack.enter_context(tc.tile_pool(name="psum", bufs=6, space="PSUM"))
        tpsum = stack.enter_context(tc.tile_pool(name="tpsum", bufs=2, space="PSUM"))

        ident_bf = const.tile([128, 128], BF16)
        make_identity(nc, ident_bf[:])
        ident_f32 = const.tile([128, 128], F32)
        make_identity(nc, ident_f32[:])
        fused_acc = const.tile([128, H], F32)

        xt_a = const.tile([128, T, 128], BF16)

        # ---------------- phase A: projections to DRAM ----------------
        RCH = 512  # rows per proj chunk (4 time steps)
        with (
            tc.tile_pool(name="wtp", bufs=1) as wtp,
            tc.tile_pool(name="xtp", bufs=3) as xtp,
            tc.tile_pool(name="evp", bufs=4) as evp,
        ):
            for m in ("c3d", "resnet"):
                if m == "resnet":
                    # audio xT transposes ride the c3d proj tail instead of
                    # delaying the first wt loads
                    for t in range(T):
                        nc.sync.dma_start_transpose(xt_a[:, t, :], x_d["audio"][t])
                dk = DIMS[m] // 128
                bp = const.tile([128, NKH], F32, tag=f"bp_{m}")
                nc.sync.dma_start(bp[:], bp_d[m].rearrange("(mo p) -> p mo", p=128))
                wt = wtp.tile([128, dk, P], BF16, tag="wt")
                wtr = wt_d[m].rearrange("(ko p) n -> p ko n", p=128)
                for k in range(dk):
                    nc.sync.dma_start(wt[:, k], wtr[:, k])
                for r in range(0, T * BS // RCH):
                    xt = xtp.tile([128, dk, RCH], BF16, tag="xt")
                    for tt in range(RCH // BS):
                        t = (r * RCH) // BS + tt
                        nc.sync.dma_start_transpose(
                            xt[:, :, tt * BS:(tt + 1) * BS], x_d[m][t])
                    for half in (0, 1):
                        pp = [psum.tile([128, 512], F32, tag="ps",
                                        name=f"pj_{m}_{r}_{half}_{mm}")
                              for mm in range(4)]
                        for k in range(dk):
                            for mm in range(4):
                                nc.tensor.matmul(
                                    pp[mm][:, 0:RCH],
                                    wt[:, k, (half * 4 + mm) * 128:
                                             (half * 4 + mm + 1) * 128],
                                    xt[:, k, :],
                                    start=(k == 0), stop=(k == dk - 1))
                        for mm in range(4):
                            mo = half * 4 + mm
                            ev = evp.tile([128, RCH], BF16, tag="ev")
                            nc.scalar.activation(
                                ev[:], pp[mm][:, 0:RCH], AF.Identity,
                                bias=bp[:, mo:mo + 1])
                            for tt in range(RCH // BS):
                                t = (r * RCH) // BS + tt
                                nc.sync.dma_start(
                                    pjt_d[m][t, mo],
                                    ev[:, tt * BS:(tt + 1) * BS])

        # ---------------- phase B: recurrences ----------------
        work = stack.enter_context(tc.tile_pool(name="work", bufs=2))
        state = stack.enter_context(tc.tile_pool(name="state", bufs=1))
        wwp = stack.enter_context(tc.tile_pool(name="wwp", bufs=1))
        pjs = stack.enter_context(tc.tile_pool(name="pjs", bufs=3))

        def recurrence(m, st_x_fn, n_kx):
            n_k = n_kx + NKH
            ww = wwp.tile([128, n_k, G4], BF16, tag="ww")
            wwr = ww_d[m].rearrange("(ko p) n -> p ko n", p=128)
            for k in range(n_k):
                nc.sync.dma_start(ww[:, k], wwr[:, k])
            if has_gate_bias:
                gb_sb = wwp.tile([128, G4], F32, tag="gb")
                nc.sync.dma_start(gb_sb[:], gb_d[m][None, :].to_broadcast([128, G4]))
            hT = state.tile([128, NKH, 128], BF16, tag="hT")
            c_st = state.tile([128, H], F32, tag="c_st")
            h_bf = state.tile([128, H], BF16, tag="h_bf")

            for t in range(T):
                def st(k):
                    return st_x_fn(t, k) if k < n_kx else hT[:, k - n_kx, :]

                ks = list(range(n_k)) if t > 0 else list(range(n_kx))
                G = [psum.tile([128, 512], F32, tag="ps", name=f"g_{m}_{t}_{n}")
                     for n in range(8)]
                for half in (0, 1):
                    for k in ks:
                        for n in range(half * 4, half * 4 + 4):
                            nc.tensor.matmul(
                                G[n][:], st(k), ww[:, k, n * 512:(n + 1) * 512],
                                start=(k == ks[0]), stop=(k == ks[-1]))

                # gate n-chunks: i: G[0:2], f: G[2:4], g: G[4:6], o: G[6:8]
                for j in (0, 1):
                    def gate_in(idx):
                        src = G[idx][:]
                        if has_gate_bias:
                            gs = work.tile([128, 512], F32, tag="gsb")
                            nc.vector.tensor_add(
                                gs[:], src, gb_sb[:, idx * 512:(idx + 1) * 512])
                            src = gs[:]
                        return src

                    sl = slice(j * 512, (j + 1) * 512)
                    sig_f = work.tile([128, 512], F32, tag="sig_f")
                    nc.scalar.activation(sig_f[:], gate_in(2 + j), AF.Sigmoid)
                    if t > 0:
                        nc.vector.tensor_mul(c_st[:, sl], sig_f[:], c_st[:, sl])
                    sig_i = work.tile([128, 512], F32, tag="sig_i")
                    nc.scalar.activation(sig_i[:], gate_in(0 + j), AF.Sigmoid)
                    tanh_g = work.tile([128, 512], F32, tag="tanh_g")
                    nc.scalar.activation(tanh_g[:], gate_in(4 + j), AF.Tanh)
                    if t > 0:
                        tmp2 = work.tile([128, 512], F32, tag="tmp2")
                        nc.vector.tensor_mul(tmp2[:], sig_i[:], tanh_g[:])
                        nc.vector.tensor_add(c_st[:, sl], c_st[:, sl], tmp2[:])
                    else:
                        nc.vector.tensor_mul(c_st[:, sl], sig_i[:], tanh_g[:])
                    tc_t = work.tile([128, 512], F32, tag="tc_t")
                    nc.scalar.activation(tc_t[:], c_st[:, sl], AF.Tanh)
                    sig_o = work.tile([128, 512], F32, tag="sig_o")
                    nc.scalar.activation(sig_o[:], gate_in(6 + j), AF.Sigmoid)
                    if t < T - 1:
                        nc.vector.tensor_mul(h_bf[:, sl], sig_o[:], tc_t[:])
                    else:
                        if m == "audio":
                            nc.vector.tensor_mul(fused_acc[:, sl], sig_o[:], tc_t[:])
                        else:
                            hf = work.tile([128, 512], F32, tag="hf")
                            nc.vector.tensor_mul(hf[:], sig_o[:], tc_t[:])
                            nc.vector.tensor_mul(
                                fused_acc[:, sl], fused_acc[:, sl], hf[:])
                if t < T - 1:
                    for k in range(NKH):
                        tp = tpsum.tile([128, 512], F32, tag="tp",
                                        name=f"tp_{m}_{t}_{k}")
                        tpv = tp[:, 0:128].bitcast(BF16)[:, 0:128]
                        nc.tensor.transpose(
                            tpv, h_bf[:, k * 128:(k + 1) * 128], ident_bf[:])
                        nc.vector.tensor_copy(hT[:, k, :], tpv)

        recurrence("audio", lambda t, k: xt_a[:, t, :], 1)

        for m in ("resnet", "c3d"):
            pjt_tiles = {}

            def stream_pjt(t, k, m=m, pjt_tiles=pjt_tiles):
                if t not in pjt_tiles:
                    pt = pjs.tile([128, NKH, BS], BF16, tag="pjt")
                    nc.sync.dma_start(
                        pt[:], pjt_d[m][t].rearrange("mo p b -> p mo b"))
                    pjt_tiles.clear()
                    pjt_tiles[t] = pt
                return pjt_tiles[t][:, k, :]

            recurrence(m, stream_pjt, NKH)

        # ---------------- final ----------------
        with tc.tile_pool(name="fin", bufs=1) as fin:
            wo = fin.tile([128, NKH, NCLS], F32)
            nc.sync.dma_start(wo[:], wout_d.rearrange("(ko p) n -> p ko n", p=128))
            ops = psum.tile([128, 512], F32, tag="ps", name="out_ps")
            for k in range(NKH):
                tp = tpsum.tile([128, 512], F32, tag="tp", name=f"ft_{k}")
                nc.tensor.transpose(
                    tp[:, 0:128], fused_acc[:, k * 128:(k + 1) * 128], ident_f32[:])
                ft = work.tile([128, 128], F32, tag="ft")
                nc.vector.tensor_copy(ft[:], tp[:, 0:128])
                nc.tensor.matmul(ops[:, 0:NCLS], ft[:], wo[:, k, :],
                                 start=(k == 0), stop=(k == NKH - 1))
            osb = work.tile([128, NCLS], F32, tag="osb")
            nc.vector.tensor_copy(osb[:], ops[:, 0:NCLS])
            nc.sync.dma_start(out_d[:], osb[:])

    nc.compile()
    return nc


def _bf16(a):
    return np.ascontiguousarray(a).astype(ml_dtypes.bfloat16)


def host_prep(inputs):
    f = np.float32
    xs = {"audio": inputs["audio_features"], "resnet": inputs["resnet_features"],
          "c3d": inputs["c3d_features"]}
    xt = {m: np.swapaxes(np.asarray(v, f), 0, 1) for m, v in xs.items()}

    wt = {"resnet": _bf16(np.asarray(inputs["W_resnet"], f).T),
          "c3d": _bf16(np.asarray(inputs["W_c3d"], f).T)}
    bp = {"resnet": np.asarray(inputs["b_resnet"], f),
          "c3d": np.asarray(inputs["b_c3d"], f)}

    dirs = {}
    has_gate_bias = False
    for d in ("fwd", "rev"):
        ww = {}
        gb = {}
        for m in MODS:
            wih = np.asarray(inputs[f"{m}_{d}_Wih"], f)
            whh = np.asarray(inputs[f"{m}_{d}_Whh"], f)
            bih = np.asarray(inputs[f"{m}_{d}_bih"], f)
            bhh = np.asarray(inputs[f"{m}_{d}_bhh"], f)
            if m == "audio":
                wa = np.asarray(inputs["W_audio"], f)
                wcomb = wih @ wa                        # (4H, AUD)
                ww[m] = _bf16(np.concatenate([wcomb.T, whh.T], axis=0))
                gb[m] = (wih @ np.asarray(inputs["b_audio"], f) + bih + bhh).astype(f)
            else:
                ww[m] = _bf16(np.concatenate([wih.T, whh.T], axis=0))
                gb[m] = (bih + bhh).astype(f)
            if np.any(gb[m] != 0):
                has_gate_bias = True
        wout_half = (np.asarray(inputs["W_out"], f)[:, :H].T if d == "fwd"
                     else np.asarray(inputs["W_out"], f)[:, H:].T)
        dirs[d] = {"ww": ww, "gb": gb, "wout": np.ascontiguousarray(wout_half)}

    in_maps = []
    for core in range(8):
        d = "fwd" if core < 4 else "rev"
        s = core % 4
        rows = slice(s * BS, (s + 1) * BS)
        im = {}
        for m in MODS:
            xm = xt[m][:, rows]
            if d == "rev":
                xm = xm[::-1]
            im[f"x_{m}"] = _bf16(xm)
            im[f"ww_{m}"] = dirs[d]["ww"][m]
            im[f"gb_{m}"] = dirs[d]["gb"][m]
        for m in ("resnet", "c3d"):
            im[f"wt_{m}"] = wt[m]
            im[f"bp_{m}"] = bp[m]
        im["wout"] = dirs[d]["wout"]
        in_maps.append(im)
    return in_maps, has_gate_bias


def assemble(results, inputs):
    out = np.zeros((B, NCLS), np.float32)
    for s in range(4):
        rows = slice(s * BS, (s + 1) * BS)
        out[rows] = results[s]["out_partial"] + results[4 + s]["out_partial"]
    out += np.asarray(inputs["b_out"], np.float32)[None, :]
    return out


def kernel(**inputs):
    global LAST_RESULTS
    in_maps, has_gate_bias = host_prep(inputs)
    nc = build_program(has_gate_bias)
    res = bass_utils.run_bass_kernel_spmd(
        nc, in_maps, core_ids=list(range(8)), trace=TRACE)
    LAST_RESULTS = res
    return assemble(res.results, inputs)



# revision 6
# speedup vs baseline: 1.7351x; 1.7351x over previous
"""Trainium2 Bass kernel for nn_Activity_Detection: 3-modality bidirectional
LSTM activity head, mixed fp8/bf16.

Sharding (8 NeuronCores): 4 batch shards (128 rows) x 2 LSTM directions.
Cores 0-3 run the forward LSTMs, cores 4-7 the reverse LSTMs on host
time-reversed features; one SPMD program.

Precision: LSTM forget gates damp errors injected at early steps, so most
matmuls run in fp8-e4m3 with DoubleRow double-pumping (~1.7x bf16 rate) and
only each direction's last few steps run in bf16:
  - projections (resnet/c3d): fp8 for t<16, bf16 for t>=16
  - x-gates: fp8 for steps <14, bf16 after (steps 14,15 upcast fp8 proj)
  - h-gates: fp8 for steps <=17, bf16 tail; h stored fp8 until step 16
All proj/gate weights are prescaled x32 (fp8 copies avoid subnormals; bf16
copies match so mixed-precision PSUM accumulation is consistent); the gate
activations descale for free via activation scale=1/32. The bf16 tail
h-weights are DMA'd into the fp8 weight buffer's space (dead by then) via a
bitcast alias to fit SBUF.
"""

import numpy as np
import ml_dtypes

import concourse.bass as bass
import concourse.bacc as bacc
import concourse.tile as tile
import concourse.mybir as mybir
from concourse.masks import make_identity
from concourse import bass_utils

E4 = mybir.dt.float8e4
BF16 = mybir.dt.bfloat16
F32 = mybir.dt.float32
AF = mybir.ActivationFunctionType
DR = mybir.MatmulPerfMode.DoubleRow

B, T = 512, 20
RES, C3D, AUD, P, H, NCLS = 2048, 4096, 128, 1024, 1024, 200
BS = 128          # batch rows per core
G4 = 4 * H        # 4096 gate dim
NKH = H // 128    # 8 h chunks
MODS = ("audio", "resnet", "c3d")
DIMS = {"audio": AUD, "resnet": RES, "c3d": C3D}

WS = 32.0         # weight prescale (both fp8 and bf16 weight copies)
SINV = 1.0 / WS
PROJ8_T = 16      # proj computed+stored fp8 for t < 16 (r-chunks 0..3)
XG8_S = 14        # x-gate matmuls fp8 for steps < 14
HMM8_S = 18       # h-gate matmuls fp8 for steps < 18
H8_STORE_S = 17   # h stored fp8 after steps < 17 (consumed by fp8 h-matmuls)

NP_E4 = ml_dtypes.float8_e4m3
NP_BF = ml_dtypes.bfloat16

TRACE = False            # set by test harness for profiling
LAST_RESULTS = None      # BassKernelResults of the last run (for profiling)


def build_program(has_gate_bias: bool):
    nc = bacc.Bacc("TRN2", target_bir_lowering=False, debug=False, num_devices=1)

    # pre-transposed inputs: resnet/c3d x as [r, p, k, 4t*128b]; audio as [p,t,b]
    x8_d = {m: nc.dram_tensor(f"x8_{m}", [4, 128, DIMS[m] // 128, 512], E4,
                              kind="ExternalInput").ap()
            for m in ("resnet", "c3d")}
    xb_d = {m: nc.dram_tensor(f"xb_{m}", [1, 128, DIMS[m] // 128, 512], BF16,
                              kind="ExternalInput").ap()
            for m in ("resnet", "c3d")}
    xa_d = nc.dram_tensor("xa", [128, T, BS], BF16, kind="ExternalInput").ap()
    wt8_d = {m: nc.dram_tensor(f"wt8_{m}", [128, DIMS[m] // 128, P], E4,
                               kind="ExternalInput").ap()
             for m in ("resnet", "c3d")}
    wtb_d = {m: nc.dram_tensor(f"wtb_{m}", [128, DIMS[m] // 128, P], BF16,
                               kind="ExternalInput").ap()
             for m in ("resnet", "c3d")}
    bp_d = {m: nc.dram_tensor(f"bp_{m}", [128, P // 128], F32,
                              kind="ExternalInput").ap()
            for m in ("resnet", "c3d")}
    # fp8 gate weights: res/c3d rows 0..7 = Wih.T, rows 8..15 = Whh.T (x32)
    ww8_d = {m: nc.dram_tensor(f"ww8_{m}", [128, 16 if m != "audio" else 8, G4],
                               E4, kind="ExternalInput").ap()
             for m in MODS}
    # bf16 gate weights (x32): res/c3d rows 0..7 = Wih.T, 8..15 = Whh.T;
    # audio row 0 = combined Wih.T, rows 1..8 = Whh.T
    wwb_d = {m: nc.dram_tensor(f"wwb_{m}", [128, 16 if m != "audio" else 9, G4],
                               BF16, kind="ExternalInput").ap()
             for m in MODS}
    gb_d = {m: nc.dram_tensor(f"gb_{m}", [G4], F32, kind="ExternalInput").ap()
            for m in MODS}
    wout_d = nc.dram_tensor("wout", [128, NKH, NCLS], F32, kind="ExternalInput").ap()
    out_d = nc.dram_tensor("out_partial", [BS, NCLS], F32, kind="ExternalOutput").ap()
    pjt8_d = {m: nc.dram_tensor(f"pjt8_{m}", [PROJ8_T, NKH, 128, BS], E4,
                                kind="Internal").ap()
              for m in ("resnet", "c3d")}
    pjtb_d = {m: nc.dram_tensor(f"pjtb_{m}", [T - PROJ8_T, NKH, 128, BS], BF16,
                                kind="Internal").ap()
              for m in ("resnet", "c3d")}

    from contextlib import ExitStack
    with tile.TileContext(nc) as tc, ExitStack() as stack:
        const = stack.enter_context(tc.tile_pool(name="const", bufs=1))
        psum = stack.enter_context(tc.tile_pool(name="psum", bufs=6, space="PSUM"))
        tpsum = stack.enter_context(tc.tile_pool(name="tpsum", bufs=2, space="PSUM"))

        ident_bf = const.tile([128, 128], BF16)
        make_identity(nc, ident_bf[:])
        ident_f32 = const.tile([128, 128], F32)
        make_identity(nc, ident_f32[:])
        fused_acc = const.tile([128, H], F32)
        xt_a = const.tile([128, T, BS], BF16)
        nc.sync.dma_start(xt_a[:], xa_d)

        # ---------------- phase A: projections to DRAM ----------------
        with (
            tc.tile_pool(name="wtp", bufs=1) as wtp,
            tc.tile_pool(name="xtp", bufs=2) as xtp,
            tc.tile_pool(name="xtpb", bufs=1) as xtpb,
            tc.tile_pool(name="evp", bufs=4) as evp,
        ):
            for m in ("c3d", "resnet"):
                dk = DIMS[m] // 128
                bp = const.tile([128, NKH], F32, tag=f"bp_{m}")
                nc.sync.dma_start(bp[:], bp_d[m])
                wt8 = wtp.tile([128, dk, P], E4, tag="wt8")
                nc.sync.dma_start(wt8[:], wt8_d[m])
                wtb = wtp.tile([128, dk, P], BF16, tag="wtb")
                nc.sync.dma_start(wtb[:], wtb_d[m])
                for r in range(5):
                    if r < 4:
                        xt = xtp.tile([128, dk, 512], E4, tag="xt8")
                        nc.sync.dma_start(xt[:], x8_d[m][r])
                    else:
                        xt = xtpb.tile([128, dk, 512], BF16, tag="xtb")
                        nc.sync.dma_start(xt[:], xb_d[m][0])
                    for half in (0, 1):
                        pp = [psum.tile([128, 512], F32, tag="ps",
                                        name=f"pj_{m}_{r}_{half}_{mm}")
                              for mm in range(4)]
                        for mm in range(4):
                            mo = half * 4 + mm
                            if r < 4:
                                for i in range(dk // 2):
                                    nc.tensor.matmul(
                                        pp[mm][:],
                                        wt8[:, 2 * i:2 * i + 2,
                                            mo * 128:(mo + 1) * 128],
                                        xt[:, 2 * i:2 * i + 2, :],
                                        start=(i == 0), stop=(i == dk // 2 - 1),
                                        perf_mode=DR)
                            else:
                                for k in range(dk):
                                    nc.tensor.matmul(
                                        pp[mm][:],
                                        wtb[:, k, mo * 128:(mo + 1) * 128],
                                        xt[:, k, :],
                                        start=(k == 0), stop=(k == dk - 1))
                        for mm in range(4):
                            mo = half * 4 + mm
                            if r < 4:
                                ev = evp.tile([128, 512], E4, tag="ev8")
                                nc.scalar.activation(
                                    ev[:], pp[mm][:], AF.Identity,
                                    bias=bp[:, mo:mo + 1], scale=SINV)
                                for tt in range(4):
                                    nc.sync.dma_start(
                                        pjt8_d[m][4 * r + tt, mo],
                                        ev[:, tt * 128:(tt + 1) * 128])
                            else:
                                ev = evp.tile([128, 512], BF16, tag="evb")
                                nc.scalar.activation(
                                    ev[:], pp[mm][:], AF.Identity,
                                    bias=bp[:, mo:mo + 1], scale=SINV)
                                for tt in range(4):
                                    nc.sync.dma_start(
                                        pjtb_d[m][tt, mo],
                                        ev[:, tt * 128:(tt + 1) * 128])

        # ---------------- phase B: recurrences ----------------
        work = stack.enter_context(tc.tile_pool(name="work", bufs=2))
        state = stack.enter_context(tc.tile_pool(name="state", bufs=1))
        wwp = stack.enter_context(tc.tile_pool(name="wwp", bufs=1))
        pjs = stack.enter_context(tc.tile_pool(name="pjs", bufs=3))
        pjsb = stack.enter_context(tc.tile_pool(name="pjsb", bufs=3))

        def recurrence(m):
            aud = m == "audio"
            # fp8 weights: [16, G4]; res/c3d: x rows 0..7, h rows 8..15.
            # audio: h rows 0..7, rows 8..15 initially unused.
            wwA = wwp.tile([128, 16, G4], E4, tag="wwA")
            if aud:
                nc.sync.dma_start(wwA[:, 0:8, :], ww8_d[m])
            else:
                nc.sync.dma_start(wwA[:], ww8_d[m])
            hb8 = 0 if aud else 8  # first fp8 h row in wwA
            # static bf16 x weights (always-bf16 audio x / res-c3d bf16 tail)
            wwB0 = wwp.tile([128, 8, G4], BF16, tag="wwB0")
            if aud:
                nc.sync.dma_start(wwB0[:, 0:1, :], wwb_d[m][:, 0:1, :])
            else:
                nc.sync.dma_start(wwB0[:], wwb_d[m][:, 0:8, :])
            # bf16 tail h-weights alias into wwA's space (dead regions)
            wwB1 = wwA[:].bitcast(BF16)  # [128, 16, 2048]
            # view row base for logical h-chunk kh:
            # res/c3d: rows 0..15 in order (x region frees at step 13, h at 17)
            # audio: kh 0..3 -> rows 8..15 (never used), kh 4..7 -> rows 0..7
            def b1_row(kh):
                if aud:
                    return 8 + 2 * kh if kh < 4 else 2 * (kh - 4)
                return 2 * kh

            def b1_slice(kh, n):
                return wwB1[:, b1_row(kh) + n // 4,
                            (n % 4) * 512:(n % 4 + 1) * 512]

            def b1_fill(khs):
                src_row = (lambda kh: 1 + kh) if aud else (lambda kh: 8 + kh)
                for kh in khs:
                    for hv in (0, 1):
                        nc.sync.dma_start(
                            wwB1[:, b1_row(kh) + hv, :],
                            wwb_d[m][:, src_row(kh), hv * 2048:(hv + 1) * 2048])

            if aud:
                b1_fill(range(0, 4))  # rows 8..15 free from the start

            if has_gate_bias:
                gb_sb = wwp.tile([128, G4], F32, tag="gb")
                nc.sync.dma_start(gb_sb[:], gb_d[m][None, :].to_broadcast([128, G4]))

            hT8 = state.tile([128, NKH, 128], E4, tag="hT8")
            hTb = state.tile([128, NKH, 128], BF16, tag="hTb")
            c_st = state.tile([128, H], F32, tag="c_st")
            h_bf = state.tile([128, H], BF16, tag="h_bf")

            pjt_cur = {}

            def load_pjt(s):
                if aud:
                    return None
                if s < XG8_S:
                    pt = pjs.tile([128, NKH, BS], E4, tag="pjt8")
                    nc.sync.dma_start(
                        pt[:], pjt8_d[m][s].rearrange("mo p b -> p mo b"))
                    return pt
                if s < PROJ8_T:
                    # upcast fp8-stored proj to bf16 (lossless)
                    pt8 = pjs.tile([128, NKH, BS], E4, tag="pjt8")
                    nc.sync.dma_start(
                        pt8[:], pjt8_d[m][s].rearrange("mo p b -> p mo b"))
                    pt = pjsb.tile([128, NKH, BS], BF16, tag="pjtb")
                    nc.vector.tensor_copy(pt[:], pt8[:])
                    return pt
                pt = pjsb.tile([128, NKH, BS], BF16, tag="pjtb")
                nc.sync.dma_start(
                    pt[:], pjtb_d[m][s - PROJ8_T].rearrange("mo p b -> p mo b"))
                return pt

            for s in range(T):
                pt = load_pjt(s)
                x_fp8 = (not aud) and s < XG8_S
                h_fp8 = s < HMM8_S

                # matmul plan: list of (stationary, moving, perf_mode) per n
                def mm_list(n):
                    out = []
                    if aud:
                        out.append((xt_a[:, s, :],
                                    wwB0[:, 0, n * 512:(n + 1) * 512], None))
                    elif x_fp8:
                        for i in range(4):
                            out.append((pt[:, 2 * i:2 * i + 2, :],
                                        wwA[:, 2 * i:2 * i + 2,
                                            n * 512:(n + 1) * 512], DR))
                    else:
                        for k in range(NKH):
                            out.append((pt[:, k, :],
                                        wwB0[:, k, n * 512:(n + 1) * 512], None))
                    if s > 0:
                        if h_fp8:
                            for i in range(4):
                                out.append((hT8[:, 2 * i:2 * i + 2, :],
                                            wwA[:, hb8 + 2 * i:hb8 + 2 * i + 2,
                                                n * 512:(n + 1) * 512], DR))
                        else:
                            for kh in range(NKH):
                                out.append((hTb[:, kh, :],
                                            b1_slice(kh, n), None))
                    return out

                G = [psum.tile([128, 512], F32, tag="ps", name=f"g_{m}_{s}_{n}")
                     for n in range(8)]
                for half in (0, 1):
                    ns = range(half * 4, half * 4 + 4)
                    plans = {n: mm_list(n) for n in ns}
                    nmm = len(plans[half * 4])
                    for idx in range(nmm):
                        for n in ns:
                            st, mv, pm = plans[n][idx]
                            nc.tensor.matmul(
                                G[n][:], st, mv,
                                start=(idx == 0), stop=(idx == nmm - 1),
                                perf_mode=pm)

                # bf16 tail h-weights fill once their alias region is dead
                if not aud and s == XG8_S - 1:
                    b1_fill(range(0, 4))      # x-fp8 rows now dead
                if s == HMM8_S - 1:
                    b1_fill(range(4, 8))      # h-fp8 rows now dead

                # gate n-chunks: i: G[0:2], f: G[2:4], g: G[4:6], o: G[6:8]
                for j in (0, 1):
                    def gate_in(idx):
                        src = G[idx][:]
                        if has_gate_bias:
                            gs = work.tile([128, 512], F32, tag="gsb")
                            nc.vector.tensor_add(
                                gs[:], src, gb_sb[:, idx * 512:(idx + 1) * 512])
                            src = gs[:]
                        return src

                    sl = slice(j * 512, (j + 1) * 512)
                    sig_f = work.tile([128, 512], F32, tag="sig_f")
                    nc.scalar.activation(sig_f[:], gate_in(2 + j), AF.Sigmoid,
                                         scale=SINV)
                    if s > 0:
                        nc.vector.tensor_mul(c_st[:, sl], sig_f[:], c_st[:, sl])
                    sig_i = work.tile([128, 512], F32, tag="sig_i")
                    nc.scalar.activation(sig_i[:], gate_in(0 + j), AF.Sigmoid,
                                         scale=SINV)
                    tanh_g = work.tile([128, 512], F32, tag="tanh_g")
                    nc.scalar.activation(tanh_g[:], gate_in(4 + j), AF.Tanh,
                                         scale=SINV)
                    if s > 0:
                        tmp2 = work.tile([128, 512], F32, tag="tmp2")
                        nc.vector.tensor_mul(tmp2[:], sig_i[:], tanh_g[:])
                        nc.vector.tensor_add(c_st[:, sl], c_st[:, sl], tmp2[:])
                    else:
                        nc.vector.tensor_mul(c_st[:, sl], sig_i[:], tanh_g[:])
                    tc_t = work.tile([128, 512], F32, tag="tc_t")
                    nc.scalar.activation(tc_t[:], c_st[:, sl], AF.Tanh)
                    sig_o = work.tile([128, 512], F32, tag="sig_o")
                    nc.scalar.activation(sig_o[:], gate_in(6 + j), AF.Sigmoid,
                                         scale=SINV)
                    if s < T - 1:
                        nc.vector.tensor_mul(h_bf[:, sl], sig_o[:], tc_t[:])
                    else:
                        if aud:
                            nc.vector.tensor_mul(fused_acc[:, sl], sig_o[:], tc_t[:])
                        else:
                            hf = work.tile([128, 512], F32, tag="hf")
                            nc.vector.tensor_mul(hf[:], sig_o[:], tc_t[:])
                            nc.vector.tensor_mul(
                                fused_acc[:, sl], fused_acc[:, sl], hf[:])
                if s < T - 1:
                    hT_dst = hT8 if s < H8_STORE_S else hTb
                    for k in range(NKH):
                        tp = tpsum.tile([128, 512], F32, tag="tp",
                                        name=f"tp_{m}_{s}_{k}")
                        tpv = tp[:, 0:64].bitcast(BF16)[:, 0:128]
                        nc.tensor.transpose(
                            tpv, h_bf[:, k * 128:(k + 1) * 128], ident_bf[:])
                        nc.vector.tensor_copy(hT_dst[:, k, :], tpv)

        for m in MODS:
            recurrence(m)

        # ---------------- final ----------------
        with tc.tile_pool(name="fin", bufs=1) as fin:
            wo = fin.tile([128, NKH, NCLS], F32)
            nc.sync.dma_start(wo[:], wout_d)
            ops = psum.tile([128, 512], F32, tag="ps", name="out_ps")
            for k in range(NKH):
                tp = tpsum.tile([128, 512], F32, tag="tp", name=f"ft_{k}")
                nc.tensor.transpose(
                    tp[:, 0:128], fused_acc[:, k * 128:(k + 1) * 128], ident_f32[:])
                ft = work.tile([128, 128], F32, tag="ft")
                nc.vector.tensor_copy(ft[:], tp[:, 0:128])
                nc.tensor.matmul(ops[:, 0:NCLS], ft[:], wo[:, k, :],
                                 start=(k == 0), stop=(k == NKH - 1))
            osb = work.tile([128, NCLS], F32, tag="osb")
            nc.vector.tensor_copy(osb[:], ops[:, 0:NCLS])
            nc.sync.dma_start(out_d[:], osb[:])

    nc.compile()
    return nc


def _f8(a):
    return np.ascontiguousarray(a).astype(NP_E4)


def _bf16(a):
    return np.ascontiguousarray(a).astype(NP_BF)


def host_prep(inputs):
    f = np.float32
    xs = {"audio": inputs["audio_features"], "resnet": inputs["resnet_features"],
          "c3d": inputs["c3d_features"]}
    xt = {m: np.swapaxes(np.asarray(v, f), 0, 1) for m, v in xs.items()}  # [T,B,D]

    wt8 = {}
    wtb = {}
    bp = {}
    for m in ("resnet", "c3d"):
        w = np.asarray(inputs[f"W_{m}"], f).T * WS      # [D, P]
        dk = DIMS[m] // 128
        wr = w.reshape(dk, 128, P).transpose(1, 0, 2)   # [128, dk, P]
        wt8[m] = _f8(wr)
        wtb[m] = _bf16(wr)
        bp[m] = np.ascontiguousarray(
            np.asarray(inputs[f"b_{m}"], f).reshape(NKH, 128).T)  # [128, 8]

    dirs = {}
    has_gate_bias = False
    for d in ("fwd", "rev"):
        ww8 = {}
        wwb = {}
        gb = {}
        for m in MODS:
            wih = np.asarray(inputs[f"{m}_{d}_Wih"], f)
            whh = np.asarray(inputs[f"{m}_{d}_Whh"], f)
            bih = np.asarray(inputs[f"{m}_{d}_bih"], f)
            bhh = np.asarray(inputs[f"{m}_{d}_bhh"], f)
            whhT = whh.T * WS                                # [H, 4H]
            whh_r = whhT.reshape(NKH, 128, G4).transpose(1, 0, 2)
            if m == "audio":
                wa = np.asarray(inputs["W_audio"], f)
                wcombT = (wih @ wa).T * WS                   # [AUD, 4H]
                ww8[m] = _f8(whh_r)                          # [128, 8, G4]
                wwb[m] = _bf16(np.concatenate(
                    [wcombT.reshape(1, 128, G4).transpose(1, 0, 2),
                     whh_r], axis=1))                        # [128, 9, G4]
                gb[m] = (WS * (wih @ np.asarray(inputs["b_audio"], f))
                         + WS * (bih + bhh)).astype(f)
            else:
                wihT = wih.T * WS                            # [P, 4H]
                wih_r = wihT.reshape(NKH, 128, G4).transpose(1, 0, 2)
                both = np.concatenate([wih_r, whh_r], axis=1)  # [128, 16, G4]
                ww8[m] = _f8(both)
                wwb[m] = _bf16(both)
                gb[m] = (WS * (bih + bhh)).astype(f)
            if np.any(gb[m] != 0):
                has_gate_bias = True
        wout_half = (np.asarray(inputs["W_out"], f)[:, :H].T if d == "fwd"
                     else np.asarray(inputs["W_out"], f)[:, H:].T)  # [H, NCLS]
        wout_r = np.ascontiguousarray(
            wout_half.reshape(NKH, 128, NCLS).transpose(1, 0, 2))
        dirs[d] = {"ww8": ww8, "wwb": wwb, "gb": gb, "wout": wout_r}

    in_maps = []
    for core in range(8):
        d = "fwd" if core < 4 else "rev"
        s = core % 4
        rows = slice(s * BS, (s + 1) * BS)
        im = {}
        for m in MODS:
            xm = xt[m][:, rows]                              # [T, BS, D]
            if d == "rev":
                xm = xm[::-1]
            if m == "audio":
                im["xa"] = _bf16(xm.transpose(2, 0, 1))      # [128, T, BS]
            else:
                dk = DIMS[m] // 128
                # [r, p, k, tt*128+b]
                xr = xm.reshape(5, 4, BS, dk, 128).transpose(0, 4, 3, 1, 2)
                xr = np.ascontiguousarray(xr.reshape(5, 128, dk, 512))
                im[f"x8_{m}"] = _f8(xr[0:4])
                im[f"xb_{m}"] = _bf16(xr[4:5])
                im[f"wt8_{m}"] = wt8[m]
                im[f"wtb_{m}"] = wtb[m]
                im[f"bp_{m}"] = bp[m]
            im[f"ww8_{m}"] = dirs[d]["ww8"][m]
            im[f"wwb_{m}"] = dirs[d]["wwb"][m]
            im[f"gb_{m}"] = dirs[d]["gb"][m]
        im["wout"] = dirs[d]["wout"]
        in_maps.append(im)
    return in_maps, has_gate_bias


def assemble(results, inputs):
    out = np.zeros((B, NCLS), np.float32)
    for s in range(4):
        rows = slice(s * BS, (s + 1) * BS)
        out[rows] = results[s]["out_partial"] + results[4 + s]["out_partial"]
    out += np.asarray(inputs["b_out"], np.float32)[None, :]
    return out


def kernel(**inputs):
    global LAST_RESULTS
    in_maps, has_gate_bias = host_prep(inputs)
    nc = build_program(has_gate_bias)
    res = bass_utils.run_bass_kernel_spmd(
        nc, in_maps, core_ids=list(range(8)), trace=TRACE)
    LAST_RESULTS = res
    return assemble(res.results, inputs)


# revision 9
# speedup vs baseline: 1.8703x; 1.0779x over previous
"""Trainium2 Bass kernel for nn_Activity_Detection: 3-modality bidirectional
LSTM activity head, mixed fp8/bf16.

Sharding (8 NeuronCores): 4 batch shards (128 rows) x 2 LSTM directions.
Cores 0-3 run the forward LSTMs, cores 4-7 the reverse LSTMs on host
time-reversed features; one SPMD program.

Precision: LSTM forget gates damp errors injected at early steps, so most
matmuls run in fp8-e4m3 with DoubleRow double-pumping (~1.7x bf16 rate) and
only each direction's last few steps run in bf16:
  - projections (resnet/c3d): fp8 for t<16, bf16 for t>=16
  - x-gates: fp8 for steps <16, bf16 after
  - h-gates: fp8 for steps <=17, bf16 tail; h stored fp8 until step 16
All proj/gate weights are prescaled x32 (fp8 copies avoid subnormals; bf16
copies match so mixed-precision PSUM accumulation is consistent); the gate
activations descale for free via activation scale=1/32. The bf16 tail
h-weights are DMA'd into the fp8 weight buffer's space (dead by then) via a
bitcast alias to fit SBUF.
"""

import numpy as np
import ml_dtypes

import concourse.bass as bass
import concourse.bacc as bacc
import concourse.tile as tile
import concourse.mybir as mybir
from concourse.masks import make_identity
from concourse import bass_utils

E4 = mybir.dt.float8e4
BF16 = mybir.dt.bfloat16
F32 = mybir.dt.float32
AF = mybir.ActivationFunctionType
DR = mybir.MatmulPerfMode.DoubleRow

B, T = 512, 20
RES, C3D, AUD, P, H, NCLS = 2048, 4096, 128, 1024, 1024, 200
BS = 128          # batch rows per core
G4 = 4 * H        # 4096 gate dim
NKH = H // 128    # 8 h chunks
MODS = ("audio", "resnet", "c3d")
DIMS = {"audio": AUD, "resnet": RES, "c3d": C3D}

WS = 32.0         # weight prescale (both fp8 and bf16 weight copies)
SINV = 1.0 / WS
PROJ8_T = 16      # proj computed+stored fp8 for t < 16 (r-chunks 0..3)
XG8_S = 16        # x-gate matmuls fp8 for steps < 16
HMM8_S = 18       # h-gate matmuls fp8 for steps < 18
H8_STORE_S = 17   # h stored fp8 after steps < 17 (consumed by fp8 h-matmuls)

NP_E4 = ml_dtypes.float8_e4m3
NP_BF = ml_dtypes.bfloat16

TRACE = False            # set by test harness for profiling
LAST_RESULTS = None      # BassKernelResults of the last run (for profiling)


def build_program(has_gate_bias: bool):
    nc = bacc.Bacc("TRN2", target_bir_lowering=False, debug=False, num_devices=1)

    # pre-transposed inputs: resnet/c3d x as [r, p, k, 4t*128b]; audio as [p,t,b]
    x8_d = {m: nc.dram_tensor(f"x8_{m}", [4, 128, DIMS[m] // 128, 512], E4,
                              kind="ExternalInput").ap()
            for m in ("resnet", "c3d")}
    xb_d = {m: nc.dram_tensor(f"xb_{m}", [1, 128, DIMS[m] // 128, 512], BF16,
                              kind="ExternalInput").ap()
            for m in ("resnet", "c3d")}
    xa_d = nc.dram_tensor("xa", [128, T, BS], BF16, kind="ExternalInput").ap()
    wt8_d = {m: nc.dram_tensor(f"wt8_{m}", [128, DIMS[m] // 128, P], E4,
                               kind="ExternalInput").ap()
             for m in ("resnet", "c3d")}
    wtb_d = {m: nc.dram_tensor(f"wtb_{m}", [128, DIMS[m] // 128, P], BF16,
                               kind="ExternalInput").ap()
             for m in ("resnet", "c3d")}
    bp_d = {m: nc.dram_tensor(f"bp_{m}", [128, P // 128], F32,
                              kind="ExternalInput").ap()
            for m in ("resnet", "c3d")}
    # fp8 gate weights: res/c3d rows 0..7 = Wih.T, rows 8..15 = Whh.T (x32)
    ww8_d = {m: nc.dram_tensor(f"ww8_{m}", [128, 16 if m != "audio" else 8, G4],
                               E4, kind="ExternalInput").ap()
             for m in MODS}
    # bf16 gate weights (x32): res/c3d rows 0..7 = Wih.T, 8..15 = Whh.T;
    # audio row 0 = combined Wih.T, rows 1..8 = Whh.T
    wwb_d = {m: nc.dram_tensor(f"wwb_{m}", [128, 16 if m != "audio" else 9, G4],
                               BF16, kind="ExternalInput").ap()
             for m in MODS}
    gb_d = {m: nc.dram_tensor(f"gb_{m}", [G4], F32, kind="ExternalInput").ap()
            for m in MODS}
    wout_d = nc.dram_tensor("wout", [128, NKH, NCLS], F32, kind="ExternalInput").ap()
    out_d = nc.dram_tensor("out_partial", [BS, NCLS], F32, kind="ExternalOutput").ap()
    pjt8_d = {m: nc.dram_tensor(f"pjt8_{m}", [PROJ8_T, NKH, 128, BS], E4,
                                kind="Internal").ap()
              for m in ("resnet", "c3d")}
    pjtb_d = {m: nc.dram_tensor(f"pjtb_{m}", [T - PROJ8_T, NKH, 128, BS], BF16,
                                kind="Internal").ap()
              for m in ("resnet", "c3d")}

    from contextlib import ExitStack
    with tile.TileContext(nc) as tc, ExitStack() as stack:
        const = stack.enter_context(tc.tile_pool(name="const", bufs=1))
        psum = stack.enter_context(tc.tile_pool(name="psum", bufs=6, space="PSUM"))
        tpsum = stack.enter_context(tc.tile_pool(name="tpsum", bufs=2, space="PSUM"))

        ident_bf = const.tile([128, 128], BF16)
        make_identity(nc, ident_bf[:])
        ident_f32 = const.tile([128, 128], F32)
        make_identity(nc, ident_f32[:])
        fused_acc = const.tile([128, H], F32)
        xt_a = const.tile([128, T, BS], BF16)
        nc.sync.dma_start(xt_a[:], xa_d)

        # ---------------- phase A: projections to DRAM ----------------
        with (
            tc.tile_pool(name="wtp", bufs=1) as wtp,
            tc.tile_pool(name="xtp", bufs=2) as xtp,
            tc.tile_pool(name="xtpb", bufs=1) as xtpb,
            tc.tile_pool(name="evp", bufs=2) as evp,
        ):
            for m in ("c3d", "resnet"):
                dk = DIMS[m] // 128
                bp = const.tile([128, NKH], F32, tag=f"bp_{m}")
                nc.sync.dma_start(bp[:], bp_d[m])
                wt8 = wtp.tile([128, dk, P], E4, tag="wt8")
                nc.sync.dma_start(wt8[:], wt8_d[m])
                wtb = wtp.tile([128, dk, P], BF16, tag="wtb")
                nc.sync.dma_start(wtb[:], wtb_d[m])
                for r in range(5):
                    if r < 4:
                        xt = xtp.tile([128, dk, 512], E4, tag="xt8")
                        nc.sync.dma_start(xt[:], x8_d[m][r])
                    else:
                        xt = xtpb.tile([128, dk, 512], BF16, tag="xtb")
                        nc.sync.dma_start(xt[:], xb_d[m][0])
                    for half in (0, 1):
                        pp = [psum.tile([128, 512], F32, tag="ps",
                                        name=f"pj_{m}_{r}_{half}_{mm}")
                              for mm in range(4)]
                        for mm in range(4):
                            mo = half * 4 + mm
                            if r < 4:
                                for i in range(dk // 2):
                                    nc.tensor.matmul(
                                        pp[mm][:],
                                        wt8[:, 2 * i:2 * i + 2,
                                            mo * 128:(mo + 1) * 128],
                                        xt[:, 2 * i:2 * i + 2, :],
                                        start=(i == 0), stop=(i == dk // 2 - 1),
                                        perf_mode=DR)
                            else:
                                for k in range(dk):
                                    nc.tensor.matmul(
                                        pp[mm][:],
                                        wtb[:, k, mo * 128:(mo + 1) * 128],
                                        xt[:, k, :],
                                        start=(k == 0), stop=(k == dk - 1))
                        for mm in range(4):
                            mo = half * 4 + mm
                            if r < 4:
                                ev = evp.tile([128, 512], E4, tag="ev8")
                                nc.scalar.activation(
                                    ev[:], pp[mm][:], AF.Identity,
                                    bias=bp[:, mo:mo + 1], scale=SINV)
                                for tt in range(4):
                                    nc.sync.dma_start(
                                        pjt8_d[m][4 * r + tt, mo],
                                        ev[:, tt * 128:(tt + 1) * 128])
                            else:
                                ev = evp.tile([128, 512], BF16, tag="evb")
                                nc.scalar.activation(
                                    ev[:], pp[mm][:], AF.Identity,
                                    bias=bp[:, mo:mo + 1], scale=SINV)
                                for tt in range(4):
                                    nc.sync.dma_start(
                                        pjtb_d[m][tt, mo],
                                        ev[:, tt * 128:(tt + 1) * 128])

        # ---------------- phase B: recurrences ----------------
        work = stack.enter_context(tc.tile_pool(name="work", bufs=2))
        state = stack.enter_context(tc.tile_pool(name="state", bufs=1))
        wwp = stack.enter_context(tc.tile_pool(name="wwp", bufs=1))
        pjs = stack.enter_context(tc.tile_pool(name="pjs", bufs=3))
        pjsb = stack.enter_context(tc.tile_pool(name="pjsb", bufs=3))

        audio_wwAh = wwp.tile([128, 8, G4], E4, tag="wwAh")
        for i in range(4):
            nc.sync.dma_start(audio_wwAh[:, 2 * i:2 * i + 2, :],
                              ww8_d["audio"][:, 2 * i:2 * i + 2, :])

        def recurrence(m, pre_wwAh=None):
            aud = m == "audio"
            # fp8 weights, split so dead regions free early for prefetch:
            # wwAx = x-side rows (res/c3d; audio: alias target only),
            # wwAh = h-side rows (all modalities).
            wwAx = wwp.tile([128, 8, G4], E4, tag="wwAx")
            if not aud:
                for i in range(4):
                    nc.sync.dma_start(wwAx[:, 2 * i:2 * i + 2, :],
                                      ww8_d[m][:, 2 * i:2 * i + 2, :])
            if pre_wwAh is not None:
                wwAh = pre_wwAh
            else:
                wwAh = wwp.tile([128, 8, G4], E4, tag="wwAh")
                base = 0 if aud else 8
                for i in range(4):
                    nc.sync.dma_start(wwAh[:, 2 * i:2 * i + 2, :],
                                      ww8_d[m][:, base + 2 * i:base + 2 * i + 2, :])
            # static bf16 x weights (always-bf16 audio x / res-c3d bf16 tail)
            wwB0 = wwp.tile([128, 8, G4], BF16, tag="wwB0")
            if aud:
                nc.sync.dma_start(wwB0[:, 0:1, :], wwb_d[m][:, 0:1, :])
            else:
                nc.sync.dma_start(wwB0[:], wwb_d[m][:, 0:8, :])
            # bf16 tail h-weights alias into wwAx/wwAh space (dead regions)
            vx = wwAx[:].bitcast(BF16)  # [128, 8, 2048]
            vh = wwAh[:].bitcast(BF16)

            def b1_slice(kh, n):
                view = vx if kh < 4 else vh
                return view[:, 2 * (kh % 4) + n // 4,
                            (n % 4) * 512:(n % 4 + 1) * 512]

            def b1_fill(khs):
                src_row = (lambda kh: 1 + kh) if aud else (lambda kh: 8 + kh)
                for kh in khs:
                    view = vx if kh < 4 else vh
                    for hv in (0, 1):
                        nc.sync.dma_start(
                            view[:, 2 * (kh % 4) + hv, :],
                            wwb_d[m][:, src_row(kh), hv * 2048:(hv + 1) * 2048])

            if aud:
                b1_fill(range(0, 4))  # wwAx unused by audio: fill anytime

            if has_gate_bias:
                gb_sb = wwp.tile([128, G4], F32, tag="gb")
                nc.sync.dma_start(gb_sb[:], gb_d[m][None, :].to_broadcast([128, G4]))

            hT8 = state.tile([128, NKH, 128], E4, tag="hT8")
            hTb = state.tile([128, NKH, 128], BF16, tag="hTb")
            c_st = state.tile([128, H], F32, tag="c_st")
            h_bf = state.tile([128, H], BF16, tag="h_bf")

            pjt_cur = {}

            def load_pjt(s):
                if aud:
                    return None
                if s < XG8_S:
                    pt = pjs.tile([128, NKH, BS], E4, tag="pjt8")
                    nc.sync.dma_start(
                        pt[:], pjt8_d[m][s].rearrange("mo p b -> p mo b"))
                    return pt
                pt = pjsb.tile([128, NKH, BS], BF16, tag="pjtb")
                nc.sync.dma_start(
                    pt[:], pjtb_d[m][s - PROJ8_T].rearrange("mo p b -> p mo b"))
                return pt

            for s in range(T):
                pt = load_pjt(s)
                x_fp8 = (not aud) and s < XG8_S
                h_fp8 = s < HMM8_S

                # matmul plan: list of (stationary, moving, perf_mode) per n
                def mm_list(n):
                    out = []
                    if aud:
                        out.append((xt_a[:, s, :],
                                    wwB0[:, 0, n * 512:(n + 1) * 512], None))
                    elif x_fp8:
                        for i in range(4):
                            out.append((pt[:, 2 * i:2 * i + 2, :],
                                        wwAx[:, 2 * i:2 * i + 2,
                                             n * 512:(n + 1) * 512], DR))
                    else:
                        for k in range(NKH):
                            out.append((pt[:, k, :],
                                        wwB0[:, k, n * 512:(n + 1) * 512], None))
                    if s > 0:
                        if h_fp8:
                            for i in range(4):
                                out.append((hT8[:, 2 * i:2 * i + 2, :],
                                            wwAh[:, 2 * i:2 * i + 2,
                                                 n * 512:(n + 1) * 512], DR))
                        else:
                            for kh in range(NKH):
                                out.append((hTb[:, kh, :],
                                            b1_slice(kh, n), None))
                    return out

                G = [psum.tile([128, 512], F32, tag="ps", name=f"g_{m}_{s}_{n}")
                     for n in range(8)]
                for half in (0, 1):
                    ns = (half, 2 + half, 4 + half, 6 + half)
                    plans = {n: mm_list(n) for n in ns}
                    nmm = len(plans[ns[0]])
                    for idx in range(nmm):
                        for n in ns:
                            st, mv, pm = plans[n][idx]
                            nc.tensor.matmul(
                                G[n][:], st, mv,
                                start=(idx == 0), stop=(idx == nmm - 1),
                                perf_mode=pm)

                # bf16 tail h-weights fill once their alias region is dead
                if not aud and s == XG8_S - 1:
                    b1_fill(range(0, 4))      # x-fp8 rows now dead
                if s == HMM8_S - 1:
                    b1_fill(range(4, 8))      # h-fp8 rows now dead

                # gate n-chunks: i: G[0:2], f: G[2:4], g: G[4:6], o: G[6:8]
                for j in (0, 1):
                    def gate_in(idx):
                        src = G[idx][:]
                        if has_gate_bias:
                            gs = work.tile([128, 512], F32, tag="gsb")
                            nc.vector.tensor_add(
                                gs[:], src, gb_sb[:, idx * 512:(idx + 1) * 512])
                            src = gs[:]
                        return src

                    sl = slice(j * 512, (j + 1) * 512)
                    sig_f = work.tile([128, 512], F32, tag="sig_f")
                    nc.scalar.activation(sig_f[:], gate_in(2 + j), AF.Sigmoid,
                                         scale=SINV)
                    if s > 0:
                        nc.vector.tensor_mul(c_st[:, sl], sig_f[:], c_st[:, sl])
                    sig_i = work.tile([128, 512], F32, tag="sig_i")
                    nc.scalar.activation(sig_i[:], gate_in(0 + j), AF.Sigmoid,
                                         scale=SINV)
                    tanh_g = work.tile([128, 512], F32, tag="tanh_g")
                    nc.scalar.activation(tanh_g[:], gate_in(4 + j), AF.Tanh,
                                         scale=SINV)
                    if s > 0:
                        tmp2 = work.tile([128, 512], F32, tag="tmp2")
                        nc.vector.tensor_mul(tmp2[:], sig_i[:], tanh_g[:])
                        nc.vector.tensor_add(c_st[:, sl], c_st[:, sl], tmp2[:])
                    else:
                        nc.vector.tensor_mul(c_st[:, sl], sig_i[:], tanh_g[:])
                    tc_t = work.tile([128, 512], F32, tag="tc_t")
                    nc.scalar.activation(tc_t[:], c_st[:, sl], AF.Tanh)
                    sig_o = work.tile([128, 512], F32, tag="sig_o")
                    nc.scalar.activation(sig_o[:], gate_in(6 + j), AF.Sigmoid,
                                         scale=SINV)
                    if s < T - 1:
                        nc.vector.tensor_mul(h_bf[:, sl], sig_o[:], tc_t[:])
                    else:
                        if aud:
                            nc.vector.tensor_mul(fused_acc[:, sl], sig_o[:], tc_t[:])
                        else:
                            hf = work.tile([128, 512], F32, tag="hf")
                            nc.vector.tensor_mul(hf[:], sig_o[:], tc_t[:])
                            nc.vector.tensor_mul(
                                fused_acc[:, sl], fused_acc[:, sl], hf[:])
                if s < T - 1:
                    hT_dst = hT8 if s < H8_STORE_S else hTb
                    for k in range(NKH):
                        tp = tpsum.tile([128, 512], F32, tag="tp",
                                        name=f"tp_{m}_{s}_{k}")
                        tpv = tp[:, 0:64].bitcast(BF16)[:, 0:128]
                        nc.tensor.transpose(
                            tpv, h_bf[:, k * 128:(k + 1) * 128], ident_bf[:])
                        nc.vector.tensor_copy(hT_dst[:, k, :], tpv)

        recurrence("audio", pre_wwAh=audio_wwAh)
        recurrence("resnet")
        recurrence("c3d")

        # ---------------- final ----------------
        with tc.tile_pool(name="fin", bufs=1) as fin:
            wo = fin.tile([128, NKH, NCLS], F32)
            nc.sync.dma_start(wo[:], wout_d)
            ops = psum.tile([128, 512], F32, tag="ps", name="out_ps")
            for k in range(NKH):
                tp = tpsum.tile([128, 512], F32, tag="tp", name=f"ft_{k}")
                nc.tensor.transpose(
                    tp[:, 0:128], fused_acc[:, k * 128:(k + 1) * 128], ident_f32[:])
                ft = work.tile([128, 128], F32, tag="ft")
                nc.vector.tensor_copy(ft[:], tp[:, 0:128])
                nc.tensor.matmul(ops[:, 0:NCLS], ft[:], wo[:, k, :],
                                 start=(k == 0), stop=(k == NKH - 1))
            osb = work.tile([128, NCLS], F32, tag="osb")
            nc.vector.tensor_copy(osb[:], ops[:, 0:NCLS])
            nc.sync.dma_start(out_d[:], osb[:])

    nc.compile()
    return nc


def _f8(a):
    return np.ascontiguousarray(a).astype(NP_E4)


def _bf16(a):
    return np.ascontiguousarray(a).astype(NP_BF)


def host_prep(inputs):
    f = np.float32
    xs = {"audio": inputs["audio_features"], "resnet": inputs["resnet_features"],
          "c3d": inputs["c3d_features"]}
    xt = {m: np.swapaxes(np.asarray(v, f), 0, 1) for m, v in xs.items()}  # [T,B,D]

    wt8 = {}
    wtb = {}
    bp = {}
    for m in ("resnet", "c3d"):
        w = np.asarray(inputs[f"W_{m}"], f).T * WS      # [D, P]
        dk = DIMS[m] // 128
        wr = w.reshape(dk, 128, P).transpose(1, 0, 2)   # [128, dk, P]
        wt8[m] = _f8(wr)
        wtb[m] = _bf16(wr)
        bp[m] = np.ascontiguousarray(
            np.asarray(inputs[f"b_{m}"], f).reshape(NKH, 128).T)  # [128, 8]

    dirs = {}
    has_gate_bias = False
    for d in ("fwd", "rev"):
        ww8 = {}
        wwb = {}
        gb = {}
        for m in MODS:
            wih = np.asarray(inputs[f"{m}_{d}_Wih"], f)
            whh = np.asarray(inputs[f"{m}_{d}_Whh"], f)
            bih = np.asarray(inputs[f"{m}_{d}_bih"], f)
            bhh = np.asarray(inputs[f"{m}_{d}_bhh"], f)
            whhT = whh.T * WS                                # [H, 4H]
            whh_r = whhT.reshape(NKH, 128, G4).transpose(1, 0, 2)
            if m == "audio":
                wa = np.asarray(inputs["W_audio"], f)
                wcombT = (wih @ wa).T * WS                   # [AUD, 4H]
                ww8[m] = _f8(whh_r)                          # [128, 8, G4]
                wwb[m] = _bf16(np.concatenate(
                    [wcombT.reshape(1, 128, G4).transpose(1, 0, 2),
                     whh_r], axis=1))                        # [128, 9, G4]
                gb[m] = (WS * (wih @ np.asarray(inputs["b_audio"], f))
                         + WS * (bih + bhh)).astype(f)
            else:
                wihT = wih.T * WS                            # [P, 4H]
                wih_r = wihT.reshape(NKH, 128, G4).transpose(1, 0, 2)
                both = np.concatenate([wih_r, whh_r], axis=1)  # [128, 16, G4]
                ww8[m] = _f8(both)
                wwb[m] = _bf16(both)
                gb[m] = (WS * (bih + bhh)).astype(f)
            if np.any(gb[m] != 0):
                has_gate_bias = True
        wout_half = (np.asarray(inputs["W_out"], f)[:, :H].T if d == "fwd"
                     else np.asarray(inputs["W_out"], f)[:, H:].T)  # [H, NCLS]
        wout_r = np.ascontiguousarray(
            wout_half.reshape(NKH, 128, NCLS).transpose(1, 0, 2))
        dirs[d] = {"ww8": ww8, "wwb": wwb, "gb": gb, "wout": wout_r}

    in_maps = []
    for core in range(8):
        d = "fwd" if core < 4 else "rev"
        s = core % 4
        rows = slice(s * BS, (s + 1) * BS)
        im = {}
        for m in MODS:
            xm = xt[m][:, rows]                              # [T, BS, D]
            if d == "rev":
                xm = xm[::-1]
            if m == "audio":
                im["xa"] = _bf16(xm.transpose(2, 0, 1))      # [128, T, BS]
            else:
                dk = DIMS[m] // 128
                # [r, p, k, tt*128+b]
                xr = xm.reshape(5, 4, BS, dk, 128).transpose(0, 4, 3, 1, 2)
                xr = np.ascontiguousarray(xr.reshape(5, 128, dk, 512))
                im[f"x8_{m}"] = _f8(xr[0:4])
                im[f"xb_{m}"] = _bf16(xr[4:5])
                im[f"wt8_{m}"] = wt8[m]
                im[f"wtb_{m}"] = wtb[m]
                im[f"bp_{m}"] = bp[m]
            im[f"ww8_{m}"] = dirs[d]["ww8"][m]
            im[f"wwb_{m}"] = dirs[d]["wwb"][m]
            im[f"gb_{m}"] = dirs[d]["gb"][m]
        im["wout"] = dirs[d]["wout"]
        in_maps.append(im)
    return in_maps, has_gate_bias


def assemble(results, inputs):
    out = np.zeros((B, NCLS), np.float32)
    for s in range(4):
        rows = slice(s * BS, (s + 1) * BS)
        out[rows] = results[s]["out_partial"] + results[4 + s]["out_partial"]
    out += np.asarray(inputs["b_out"], np.float32)[None, :]
    return out


def kernel(**inputs):
    global LAST_RESULTS
    in_maps, has_gate_bias = host_prep(inputs)
    nc = build_program(has_gate_bias)
    res = bass_utils.run_bass_kernel_spmd(
        nc, in_maps, core_ids=list(range(8)), trace=TRACE)
    LAST_RESULTS = res
    return assemble(res.results, inputs)


# revision 11
# speedup vs baseline: 1.9704x; 1.0536x over previous
"""Trainium2 Bass kernel for nn_Activity_Detection: 3-modality bidirectional
LSTM activity head, mixed fp8/bf16.

Sharding (8 NeuronCores): 4 batch shards (128 rows) x 2 LSTM directions.
Cores 0-3 run the forward LSTMs, cores 4-7 the reverse LSTMs on host
time-reversed features; one SPMD program.

Precision: LSTM forget gates damp errors injected at early steps, so most
matmuls run in fp8-e4m3 with DoubleRow double-pumping (~1.7x bf16 rate) and
only each direction's last few steps run in bf16:
  - projections (resnet/c3d): fp8 for t<16, bf16 for t>=16
  - x-gates: fp8 for steps <16, bf16 after
  - h-gates: fp8 for steps <=17, bf16 tail; h stored fp8 until step 16
All proj/gate weights are prescaled x32 (fp8 copies avoid subnormals; bf16
copies match so mixed-precision PSUM accumulation is consistent); the gate
activations descale for free via activation scale=1/32. The bf16 tail
h-weights are DMA'd into the fp8 weight buffer's space (dead by then) via a
bitcast alias to fit SBUF.
"""

import numpy as np
import ml_dtypes

import concourse.bass as bass
import concourse.bacc as bacc
import concourse.tile as tile
import concourse.mybir as mybir
from concourse.masks import make_identity
from concourse import bass_utils

E4 = mybir.dt.float8e4
BF16 = mybir.dt.bfloat16
F32 = mybir.dt.float32
AF = mybir.ActivationFunctionType
DR = mybir.MatmulPerfMode.DoubleRow

B, T = 512, 20
RES, C3D, AUD, P, H, NCLS = 2048, 4096, 128, 1024, 1024, 200
BS = 128          # batch rows per core
G4 = 4 * H        # 4096 gate dim
NKH = H // 128    # 8 h chunks
MODS = ("audio", "resnet", "c3d")
DIMS = {"audio": AUD, "resnet": RES, "c3d": C3D}

WS = 32.0         # weight prescale (both fp8 and bf16 weight copies)
SINV = 1.0 / WS
PROJ8_T = 16      # proj computed+stored fp8 for t < 16 (r-chunks 0..3)
XG8_S = 16        # x-gate matmuls fp8 for steps < 16
HMM8_S = 18       # h-gate matmuls fp8 for steps < 18
H8_STORE_S = 17   # h stored fp8 after steps < 17 (consumed by fp8 h-matmuls)

NP_E4 = ml_dtypes.float8_e4m3
NP_BF = ml_dtypes.bfloat16

TRACE = False            # set by test harness for profiling
LAST_RESULTS = None      # BassKernelResults of the last run (for profiling)


def build_program(has_gate_bias: bool):
    nc = bacc.Bacc("TRN2", target_bir_lowering=False, debug=False, num_devices=1)

    # pre-transposed inputs: resnet/c3d x as [r, p, k, 4t*128b]; audio as [p,t,b]
    x8_d = {m: nc.dram_tensor(f"x8_{m}", [4, 128, DIMS[m] // 128, 512], E4,
                              kind="ExternalInput").ap()
            for m in ("resnet", "c3d")}
    xb_d = {m: nc.dram_tensor(f"xb_{m}", [1, 128, DIMS[m] // 128, 512], BF16,
                              kind="ExternalInput").ap()
            for m in ("resnet", "c3d")}
    xa_d = nc.dram_tensor("xa", [128, T, BS], BF16, kind="ExternalInput").ap()
    wt8_d = {m: nc.dram_tensor(f"wt8_{m}", [128, DIMS[m] // 128, P], E4,
                               kind="ExternalInput").ap()
             for m in ("resnet", "c3d")}
    wtb_d = {m: nc.dram_tensor(f"wtb_{m}", [128, DIMS[m] // 128, P], BF16,
                               kind="ExternalInput").ap()
             for m in ("resnet", "c3d")}
    bp_d = {m: nc.dram_tensor(f"bp_{m}", [128, P // 128], F32,
                              kind="ExternalInput").ap()
            for m in ("resnet", "c3d")}
    # fp8 gate weights: res/c3d rows 0..7 = Wih.T, rows 8..15 = Whh.T (x32)
    ww8_d = {m: nc.dram_tensor(f"ww8_{m}", [128, 16 if m != "audio" else 8, G4],
                               E4, kind="ExternalInput").ap()
             for m in MODS}
    # bf16 gate weights (x32): res/c3d rows 0..7 = Wih.T, 8..15 = Whh.T;
    # audio row 0 = combined Wih.T, rows 1..8 = Whh.T
    wwb_d = {m: nc.dram_tensor(f"wwb_{m}", [128, 16 if m != "audio" else 9, G4],
                               BF16, kind="ExternalInput").ap()
             for m in MODS}
    gb_d = {m: nc.dram_tensor(f"gb_{m}", [G4], F32, kind="ExternalInput").ap()
            for m in MODS}
    wout_d = nc.dram_tensor("wout", [128, NKH, NCLS], F32, kind="ExternalInput").ap()
    out_d = nc.dram_tensor("out_partial", [BS, NCLS], F32, kind="ExternalOutput").ap()
    pjt8_d = {m: nc.dram_tensor(f"pjt8_{m}", [PROJ8_T, NKH, 128, BS], E4,
                                kind="Internal").ap()
              for m in ("resnet", "c3d")}
    pjtb_d = {m: nc.dram_tensor(f"pjtb_{m}", [T - PROJ8_T, NKH, 128, BS], BF16,
                                kind="Internal").ap()
              for m in ("resnet", "c3d")}

    from contextlib import ExitStack
    with tile.TileContext(nc) as tc, ExitStack() as stack:
        const = stack.enter_context(tc.tile_pool(name="const", bufs=1))
        psum = stack.enter_context(tc.tile_pool(name="psum", bufs=6, space="PSUM"))
        tpsum = stack.enter_context(tc.tile_pool(name="tpsum", bufs=2, space="PSUM"))

        ident_bf = const.tile([128, 128], BF16)
        make_identity(nc, ident_bf[:])
        xt_a = const.tile([128, T, BS], BF16)
        nc.sync.dma_start(xt_a[:], xa_d)

        # ---------------- phase A: projections to DRAM ----------------
        with (
            tc.tile_pool(name="wtp", bufs=1) as wtp,
            tc.tile_pool(name="xtp", bufs=2) as xtp,
            tc.tile_pool(name="xtpb", bufs=1) as xtpb,
            tc.tile_pool(name="evp", bufs=2) as evp,
        ):
            for m in ("c3d", "resnet"):
                dk = DIMS[m] // 128
                bp = const.tile([128, NKH], F32, tag=f"bp_{m}")
                nc.sync.dma_start(bp[:], bp_d[m])
                wt8 = wtp.tile([128, dk, P], E4, tag="wt8")
                nc.sync.dma_start(wt8[:], wt8_d[m])
                wtb = wtp.tile([128, dk, P], BF16, tag="wtb")
                nc.sync.dma_start(wtb[:], wtb_d[m])
                for r in range(5):
                    if r < 4:
                        xt = xtp.tile([128, dk, 512], E4, tag="xt8")
                        nc.sync.dma_start(xt[:], x8_d[m][r])
                    else:
                        xt = xtpb.tile([128, dk, 512], BF16, tag="xtb")
                        nc.sync.dma_start(xt[:], xb_d[m][0])
                    for half in (0, 1):
                        pp = [psum.tile([128, 512], F32, tag="ps",
                                        name=f"pj_{m}_{r}_{half}_{mm}")
                              for mm in range(4)]
                        for mm in range(4):
                            mo = half * 4 + mm
                            if r < 4:
                                for i in range(dk // 2):
                                    nc.tensor.matmul(
                                        pp[mm][:],
                                        wt8[:, 2 * i:2 * i + 2,
                                            mo * 128:(mo + 1) * 128],
                                        xt[:, 2 * i:2 * i + 2, :],
                                        start=(i == 0), stop=(i == dk // 2 - 1),
                                        perf_mode=DR)
                            else:
                                for k in range(dk):
                                    nc.tensor.matmul(
                                        pp[mm][:],
                                        wtb[:, k, mo * 128:(mo + 1) * 128],
                                        xt[:, k, :],
                                        start=(k == 0), stop=(k == dk - 1))
                        for mm in range(4):
                            mo = half * 4 + mm
                            if r < 4:
                                ev = evp.tile([128, 512], E4, tag="ev8")
                                nc.scalar.activation(
                                    ev[:], pp[mm][:], AF.Identity,
                                    bias=bp[:, mo:mo + 1], scale=SINV)
                                for tt in range(4):
                                    nc.sync.dma_start(
                                        pjt8_d[m][4 * r + tt, mo],
                                        ev[:, tt * 128:(tt + 1) * 128])
                            else:
                                ev = evp.tile([128, 512], BF16, tag="evb")
                                nc.scalar.activation(
                                    ev[:], pp[mm][:], AF.Identity,
                                    bias=bp[:, mo:mo + 1], scale=SINV)
                                for tt in range(4):
                                    nc.sync.dma_start(
                                        pjtb_d[m][tt, mo],
                                        ev[:, tt * 128:(tt + 1) * 128])

        # ---------------- phase B: recurrences ----------------
        work = stack.enter_context(tc.tile_pool(name="work", bufs=2))
        state = stack.enter_context(tc.tile_pool(name="state", bufs=1))
        wwp = stack.enter_context(tc.tile_pool(name="wwp", bufs=1))
        pjs = stack.enter_context(tc.tile_pool(name="pjs", bufs=3))
        pjsb = stack.enter_context(tc.tile_pool(name="pjsb", bufs=3))

        audio_wwAh = wwp.tile([128, 8, G4], E4, tag="wwAh")
        for i in range(4):
            nc.sync.dma_start(audio_wwAh[:, 2 * i:2 * i + 2, :],
                              ww8_d["audio"][:, 2 * i:2 * i + 2, :])

        def recurrence(m, pre_wwAh=None):
            aud = m == "audio"
            # fp8 weights, split so dead regions free early for prefetch:
            # wwAx = x-side rows (res/c3d; audio: alias target only),
            # wwAh = h-side rows (all modalities).
            wwAx = wwp.tile([128, 8, G4], E4, tag="wwAx")
            if not aud:
                for i in range(4):
                    nc.sync.dma_start(wwAx[:, 2 * i:2 * i + 2, :],
                                      ww8_d[m][:, 2 * i:2 * i + 2, :])
            if pre_wwAh is not None:
                wwAh = pre_wwAh
            else:
                wwAh = wwp.tile([128, 8, G4], E4, tag="wwAh")
                base = 0 if aud else 8
                for i in range(4):
                    nc.sync.dma_start(wwAh[:, 2 * i:2 * i + 2, :],
                                      ww8_d[m][:, base + 2 * i:base + 2 * i + 2, :])
            # static bf16 x weights (always-bf16 audio x / res-c3d bf16 tail)
            wwB0 = wwp.tile([128, 8, G4], BF16, tag="wwB0")
            if aud:
                nc.sync.dma_start(wwB0[:, 0:1, :], wwb_d[m][:, 0:1, :])
            else:
                nc.sync.dma_start(wwB0[:], wwb_d[m][:, 0:8, :])
            # bf16 tail h-weights alias into wwAx/wwAh space (dead regions)
            vx = wwAx[:].bitcast(BF16)  # [128, 8, 2048]
            vh = wwAh[:].bitcast(BF16)

            def b1_slice(kh, n):
                view = vx if kh < 4 else vh
                return view[:, 2 * (kh % 4) + n // 4,
                            (n % 4) * 512:(n % 4 + 1) * 512]

            def b1_fill(khs):
                src_row = (lambda kh: 1 + kh) if aud else (lambda kh: 8 + kh)
                for kh in khs:
                    view = vx if kh < 4 else vh
                    for hv in (0, 1):
                        nc.sync.dma_start(
                            view[:, 2 * (kh % 4) + hv, :],
                            wwb_d[m][:, src_row(kh), hv * 2048:(hv + 1) * 2048])

            if aud:
                b1_fill(range(0, 4))  # wwAx unused by audio: fill anytime

            if has_gate_bias:
                gb_sb = wwp.tile([128, G4], F32, tag="gb")
                nc.sync.dma_start(gb_sb[:], gb_d[m][None, :].to_broadcast([128, G4]))

            stt = {}

            def ensure_state():
                if not stt:
                    stt["hT8"] = state.tile([128, NKH, 128], E4, tag="hT8",
                                            name=f"hT8_{m}")
                    stt["hTb"] = state.tile([128, NKH, 128], BF16, tag="hTb",
                                            name=f"hTb_{m}")
                    stt["c_st"] = state.tile([128, H], F32, tag="c_st",
                                             name=f"c_st_{m}")
                    stt["h_bf"] = state.tile([128, H], BF16, tag="h_bf",
                                             name=f"h_bf_{m}")

            pjt_cur = {}

            def load_pjt(s):
                if aud:
                    return None
                if s < XG8_S:
                    pt = pjs.tile([128, NKH, BS], E4, tag="pjt8")
                    nc.sync.dma_start(
                        pt[:], pjt8_d[m][s].rearrange("mo p b -> p mo b"))
                    return pt
                pt = pjsb.tile([128, NKH, BS], BF16, tag="pjtb")
                nc.sync.dma_start(
                    pt[:], pjtb_d[m][s - PROJ8_T].rearrange("mo p b -> p mo b"))
                return pt

            for s in range(T):
                pt = load_pjt(s)
                x_fp8 = (not aud) and s < XG8_S
                h_fp8 = s < HMM8_S

                # matmul plan: list of (stationary, moving, perf_mode) per n
                def mm_list(n):
                    out = []
                    if aud:
                        out.append((xt_a[:, s, :],
                                    wwB0[:, 0, n * 512:(n + 1) * 512], None))
                    elif x_fp8:
                        for i in range(4):
                            out.append((pt[:, 2 * i:2 * i + 2, :],
                                        wwAx[:, 2 * i:2 * i + 2,
                                             n * 512:(n + 1) * 512], DR))
                    else:
                        for k in range(NKH):
                            out.append((pt[:, k, :],
                                        wwB0[:, k, n * 512:(n + 1) * 512], None))
                    if s > 0:
                        if h_fp8:
                            for i in range(4):
                                out.append((hT8[:, 2 * i:2 * i + 2, :],
                                            wwAh[:, 2 * i:2 * i + 2,
                                                 n * 512:(n + 1) * 512], DR))
                        else:
                            for kh in range(NKH):
                                out.append((hTb[:, kh, :],
                                            b1_slice(kh, n), None))
                    return out

                G = [psum.tile([128, 512], F32, tag="ps", name=f"g_{m}_{s}_{n}")
                     for n in range(8)]
                for half in (0, 1):
                    ns = (half, 2 + half, 4 + half, 6 + half)
                    plans = {n: mm_list(n) for n in ns}
                    nmm = len(plans[ns[0]])
                    for idx in range(nmm):
                        for n in ns:
                            st, mv, pm = plans[n][idx]
                            nc.tensor.matmul(
                                G[n][:], st, mv,
                                start=(idx == 0), stop=(idx == nmm - 1),
                                perf_mode=pm)

                # bf16 tail h-weights fill once their alias region is dead
                if not aud and s == XG8_S - 1:
                    b1_fill(range(0, 4))      # x-fp8 rows now dead
                if s == HMM8_S - 1:
                    b1_fill(range(4, 8))      # h-fp8 rows now dead

                # gate n-chunks: i: G[0:2], f: G[2:4], g: G[4:6], o: G[6:8]
                for j in (0, 1):
                    def gate_in(idx):
                        src = G[idx][:]
                        if has_gate_bias:
                            gs = work.tile([128, 512], F32, tag="gsb")
                            nc.vector.tensor_add(
                                gs[:], src, gb_sb[:, idx * 512:(idx + 1) * 512])
                            src = gs[:]
                        return src

                    sl = slice(j * 512, (j + 1) * 512)
                    sig_f = work.tile([128, 512], F32, tag="sig_f")
                    nc.scalar.activation(sig_f[:], gate_in(2 + j), AF.Sigmoid,
                                         scale=SINV)
                    if s > 0:
                        nc.vector.tensor_mul(c_st[:, sl], sig_f[:], c_st[:, sl])
                    sig_i = work.tile([128, 512], F32, tag="sig_i")
                    nc.scalar.activation(sig_i[:], gate_in(0 + j), AF.Sigmoid,
                                         scale=SINV)
                    tanh_g = work.tile([128, 512], F32, tag="tanh_g")
                    nc.scalar.activation(tanh_g[:], gate_in(4 + j), AF.Tanh,
                                         scale=SINV)
                    if s > 0:
                        tmp2 = work.tile([128, 512], F32, tag="tmp2")
                        nc.vector.tensor_mul(tmp2[:], sig_i[:], tanh_g[:])
                        nc.vector.tensor_add(c_st[:, sl], c_st[:, sl], tmp2[:])
                    else:
                        nc.vector.tensor_mul(c_st[:, sl], sig_i[:], tanh_g[:])
                    tc_t = work.tile([128, 512], F32, tag="tc_t")
                    nc.scalar.activation(tc_t[:], c_st[:, sl], AF.Tanh)
                    sig_o = work.tile([128, 512], F32, tag="sig_o")
                    nc.scalar.activation(sig_o[:], gate_in(6 + j), AF.Sigmoid,
                                         scale=SINV)
                    if s < T - 1:
                        nc.vector.tensor_mul(h_bf[:, sl], sig_o[:], tc_t[:])
                    else:
                        if aud:
                            nc.vector.tensor_mul(fused_acc[:, sl], sig_o[:], tc_t[:])
                        else:
                            hf = work.tile([128, 512], F32, tag="hf")
                            nc.vector.tensor_mul(hf[:], sig_o[:], tc_t[:])
                            nc.vector.tensor_mul(
                                fused_acc[:, sl], fused_acc[:, sl], hf[:])
                if s < T - 1:
                    hT_dst = hT8 if s < H8_STORE_S else hTb
                    for k in range(NKH):
                        tp = tpsum.tile([128, 512], F32, tag="tp",
                                        name=f"tp_{m}_{s}_{k}")
                        tpv = tp[:, 0:64].bitcast(BF16)[:, 0:128]
                        nc.tensor.transpose(
                            tpv, h_bf[:, k * 128:(k + 1) * 128], ident_bf[:])
                        nc.vector.tensor_copy(hT_dst[:, k, :], tpv)

        recurrence("audio", pre_wwAh=audio_wwAh)
        recurrence("resnet")
        recurrence("c3d")

        # ---------------- final ----------------
        with tc.tile_pool(name="fin", bufs=1) as fin:
            wo = fin.tile([128, NKH, NCLS], F32)
            nc.sync.dma_start(wo[:], wout_d)
            ops = psum.tile([128, 512], F32, tag="ps", name="out_ps")
            for k in range(NKH):
                tp = tpsum.tile([128, 512], F32, tag="tp", name=f"ft_{k}")
                nc.tensor.transpose(
                    tp[:, 0:128], fused_acc[:, k * 128:(k + 1) * 128], ident_f32[:])
                ft = work.tile([128, 128], F32, tag="ft")
                nc.vector.tensor_copy(ft[:], tp[:, 0:128])
                nc.tensor.matmul(ops[:, 0:NCLS], ft[:], wo[:, k, :],
                                 start=(k == 0), stop=(k == NKH - 1))
            osb = work.tile([128, NCLS], F32, tag="osb")
            nc.vector.tensor_copy(osb[:], ops[:, 0:NCLS])
            nc.sync.dma_start(out_d[:], osb[:])

    nc.compile()
    return nc


def _f8(a):
    return np.ascontiguousarray(a).astype(NP_E4)


def _bf16(a):
    return np.ascontiguousarray(a).astype(NP_BF)


def host_prep(inputs):
    f = np.float32
    xs = {"audio": inputs["audio_features"], "resnet": inputs["resnet_features"],
          "c3d": inputs["c3d_features"]}
    xt = {m: np.swapaxes(np.asarray(v, f), 0, 1) for m, v in xs.items()}  # [T,B,D]

    wt8 = {}
    wtb = {}
    bp = {}
    for m in ("resnet", "c3d"):
        w = np.asarray(inputs[f"W_{m}"], f).T * WS      # [D, P]
        dk = DIMS[m] // 128
        wr = w.reshape(dk, 128, P).transpose(1, 0, 2)   # [128, dk, P]
        wt8[m] = _f8(wr)
        wtb[m] = _bf16(wr)
        bp[m] = np.ascontiguousarray(
            np.asarray(inputs[f"b_{m}"], f).reshape(NKH, 128).T)  # [128, 8]

    dirs = {}
    has_gate_bias = False
    for d in ("fwd", "rev"):
        ww8 = {}
        wwb = {}
        gb = {}
        for m in MODS:
            wih = np.asarray(inputs[f"{m}_{d}_Wih"], f)
            whh = np.asarray(inputs[f"{m}_{d}_Whh"], f)
            bih = np.asarray(inputs[f"{m}_{d}_bih"], f)
            bhh = np.asarray(inputs[f"{m}_{d}_bhh"], f)
            whhT = whh.T * WS                                # [H, 4H]
            whh_r = whhT.reshape(NKH, 128, G4).transpose(1, 0, 2)
            if m == "audio":
                wa = np.asarray(inputs["W_audio"], f)
                wcombT = (wih @ wa).T * WS                   # [AUD, 4H]
                ww8[m] = _f8(whh_r)                          # [128, 8, G4]
                wwb[m] = _bf16(np.concatenate(
                    [wcombT.reshape(1, 128, G4).transpose(1, 0, 2),
                     whh_r], axis=1))                        # [128, 9, G4]
                gb[m] = (WS * (wih @ np.asarray(inputs["b_audio"], f))
                         + WS * (bih + bhh)).astype(f)
            else:
                wihT = wih.T * WS                            # [P, 4H]
                wih_r = wihT.reshape(NKH, 128, G4).transpose(1, 0, 2)
                both = np.concatenate([wih_r, whh_r], axis=1)  # [128, 16, G4]
                ww8[m] = _f8(both)
                wwb[m] = _bf16(both)
                gb[m] = (WS * (bih + bhh)).astype(f)
            if np.any(gb[m] != 0):
                has_gate_bias = True
        wout_half = (np.asarray(inputs["W_out"], f)[:, :H].T if d == "fwd"
                     else np.asarray(inputs["W_out"], f)[:, H:].T)  # [H, NCLS]
        wout_r = np.ascontiguousarray(
            wout_half.reshape(NKH, 128, NCLS).transpose(1, 0, 2))
        dirs[d] = {"ww8": ww8, "wwb": wwb, "gb": gb, "wout": wout_r}

    in_maps = []
    for core in range(8):
        d = "fwd" if core < 4 else "rev"
        s = core % 4
        rows = slice(s * BS, (s + 1) * BS)
        im = {}
        for m in MODS:
            xm = xt[m][:, rows]                              # [T, BS, D]
            if d == "rev":
                xm = xm[::-1]
            if m == "audio":
                im["xa"] = _bf16(xm.transpose(2, 0, 1))      # [128, T, BS]
            else:
                dk = DIMS[m] // 128
                # [r, p, k, tt*128+b]
                xr = xm.reshape(5, 4, BS, dk, 128).transpose(0, 4, 3, 1, 2)
                xr = np.ascontiguousarray(xr.reshape(5, 128, dk, 512))
                im[f"x8_{m}"] = _f8(xr[0:4])
                im[f"xb_{m}"] = _bf16(xr[4:5])
                im[f"wt8_{m}"] = wt8[m]
                im[f"wtb_{m}"] = wtb[m]
                im[f"bp_{m}"] = bp[m]
            im[f"ww8_{m}"] = dirs[d]["ww8"][m]
            im[f"wwb_{m}"] = dirs[d]["wwb"][m]
            im[f"gb_{m}"] = dirs[d]["gb"][m]
        im["wout"] = dirs[d]["wout"]
        in_maps.append(im)
    return in_maps, has_gate_bias


def assemble(results, inputs):
    out = np.zeros((B, NCLS), np.float32)
    for s in range(4):
        rows = slice(s * BS, (s + 1) * BS)
        out[rows] = results[s]["out_partial"] + results[4 + s]["out_partial"]
    out += np.asarray(inputs["b_out"], np.float32)[None, :]
    return out


def kernel(**inputs):
    global LAST_RESULTS
    in_maps, has_gate_bias = host_prep(inputs)
    nc = build_program(has_gate_bias)
    res = bass_utils.run_bass_kernel_spmd(
        nc, in_maps, core_ids=list(range(8)), trace=TRACE)
    LAST_RESULTS = res
    return assemble(res.results, inputs)
